# revision 37
# baseline (speedup 1.0000x reference)
"""Causal multi-head attention (B=4, S=2048, E=1024, H=16, K=64, fp32) on 8 TRN2 cores.

Sharding: core c = 2*b + g handles batch b and head-group g (8 of 16 heads).
Each core computes QKV projections for its heads, causal flash-style
attention, and a partial output projection; the host sums the two
head-group partials per batch.

Layout choices (all on-chip, per core):
  - Projections produce Q^T, K^T directly ([hk, tok], hk on partitions) so
    QK^T needs no transposes; logits are computed transposed ([m, q], keys on
    partitions), which also lets PV consume the probs tile directly as the
    matmul moving side with no transpose anywhere.
  - Softmax skips max-subtraction (logits ~ N(0,1); exp is safe in fp32) and
    gets the normalizer for free from 64 ones-columns appended to V: the PV
    matmul writes O^T to psum rows 0:64 and replicates the per-q prob-sums
    across rows 64:128, so normalization is a plain reciprocal+multiply with
    no cross-partition broadcast.
  - Causal masking multiplies the two diagonal-band blocks of each q-chunk by
    a 0/1 mask after exp.
  - All matmuls run in bf16 (fp32 PSUM accumulate).
  - q-chunks are processed in pairs and projections are loop-ordered so that
    consecutive PE matmuls share their stationary operand; a post-pass
    removes the duplicate LDWEIGHTS.  Projections for head-pair hp+1 are
    emitted between attention chunks of hp so the (ACT-bound) softmax
    stretches keep the PE busy and the HAM clock-gate stays warm.
"""

import numpy as np
import ml_dtypes

S = 2048          # sequence length
E = 1024          # model dim
HG = 8            # heads per core
HK = HG * 64      # packed head*K = 512
QC = 256          # q-chunk
NP = S // (2 * QC)  # 4 q-chunk pairs
BF16 = ml_dtypes.bfloat16

_compiled = None


def _legalize_sync_waits(nc, max_other=1, max_evsem=2):
    """Walrus in this toolchain caps sync-wait commands per instruction; split
    excess waits onto standalone InstNoOps on the same engine."""
    import concourse.mybir as mybir

    for fn in nc.m.functions:
        for bb in fn.blocks:
            new = []
            changed = False
            for inst in bb.instructions:
                si = inst.sync_info
                ow = list(si.on_wait) if (si is not None and si.on_wait) else []
                cap = (
                    max_evsem
                    if type(inst).__name__ == "InstEventSemaphore"
                    else max_other
                )
                if len(ow) > cap:
                    extras, kept = ow[:-cap], ow[-cap:]
                    for i in range(0, len(extras), max_other):
                        new.append(
                            mybir.InstNoOp(
                                name=f"{inst.name}-waitsplit-{i}",
                                sync_info=mybir.SyncInfo(
                                    on_wait=extras[i : i + max_other], on_update=[]
                                ),
                                engine=inst.engine,
                                bass_nofuse=True,
                            )
                        )
                    si.on_wait = kept
                    changed = True
                new.append(inst)
            if changed:
                bb.instructions = new


def _dedup_ldweights(nc):
    """Remove an InstLdweights whose weight AP is identical to the immediately
    preceding PE weight load (pattern LDW A, MM, LDW A, MM ...).  All dedup
    targets here load from persistent SBUF tensors, so there is no WAR hazard;
    any waits/updates on the removed load migrate to the next PE
    instruction."""
    import concourse.mybir as mybir
    import re as _re

    n = 0
    for fn in nc.m.functions:
        for bb in fn.blocks:
            insts = bb.instructions
            last_key = None
            drop = set()
            pend_w, pend_u = [], []
            for idx, inst in enumerate(insts):
                eng = getattr(inst, "engine", None)
                if eng != mybir.EngineType.PE:
                    continue
                nm = type(inst).__name__
                if pend_w or pend_u:
                    si = inst.sync_info
                    w = (list(si.on_wait or []) if si else []) + pend_w
                    u = (list(si.on_update or []) if si else []) + pend_u
                    inst.sync_info = mybir.SyncInfo(on_wait=w, on_update=u)
                    pend_w, pend_u = [], []
                if nm == "InstLdweights":
                    js = mybir.instruction_to_pretty_json_string(inst)
                    key = _re.sub(r'"name":\s*"[^"]*",?', "", js)
                    key = _re.sub(r'"sync_info":\s*\{.*?\n  \}', "", key, flags=_re.S)
                    key = _re.sub(r'"debug[^"]*":\s*(\{.*?\n  \}|"[^"]*"|null),?', "", key, flags=_re.S)
                    if key == last_key:
                        si = inst.sync_info
                        if si is not None:
                            pend_w += list(si.on_wait or [])
                            pend_u += list(si.on_update or [])
                        drop.add(idx)
                        n += 1
                    else:
                        last_key = key
                elif nm != "InstMatmult":
                    last_key = None
            if drop:
                bb.instructions = [i for j, i in enumerate(insts) if j not in drop]
    return n


def _build_nc(legalize=True):
    import concourse.bass as bass
    import concourse.mybir as mybir
    import concourse.tile as tile

    bf16 = mybir.dt.bfloat16
    f32 = mybir.dt.float32
    Exp = mybir.ActivationFunctionType.Exp
    mult = mybir.AluOpType.mult

    nc = bass.Bass("TRN2", target_bir_lowering=False, debug=False)

    xT = nc.dram_tensor("xT", [E, S], bf16, kind="ExternalInput")
    wq = nc.dram_tensor("wq", [E, HK], bf16, kind="ExternalInput")
    wk = nc.dram_tensor("wk", [E, HK], bf16, kind="ExternalInput")
    wv = nc.dram_tensor("wv", [E, HK], bf16, kind="ExternalInput")
    wo = nc.dram_tensor("wo", [HK, E], bf16, kind="ExternalInput")
    masks = nc.dram_tensor("masks", [128, 2, QC], bf16, kind="ExternalInput")
    yT = nc.dram_tensor("yT", [E, S], bf16, kind="ExternalOutput")

    with tile.TileContext(nc) as tc:
        with (
            tc.tile_pool(name="persist", bufs=1) as persist,
            tc.tile_pool(name="pt", bufs=3) as pt_pool,
            tc.tile_pool(name="small", bufs=6) as small,
            tc.tile_pool(name="ysb", bufs=2) as ypool,
            tc.tile_pool(name="ps", bufs=3, space="PSUM") as ps_shared,
            tc.tile_pool(name="ps_pv", bufs=2, space="PSUM") as ps_pv,
        ):
            # ---- load inputs ----
            # All input DMAs share the sync-engine HW queue, so they transfer
            # in issue order: interleave the wv/x chunks the V projection
            # consumes first, then wq/wk (needed ~25us in), masks, and wo
            # (needed only ~170us in).  This lets the PE start ~2us after
            # launch instead of waiting for the whole 8MB input load.
            xT_r = xT.ap().rearrange("(ec p) t -> p ec t", p=128)
            wv_r = wv.ap().rearrange("(ec p) f -> p ec f", p=128)
            xts = []
            wvs = []
            for ec in range(8):
                wvc = persist.tile([128, HK], bf16, name=f"wv{ec}")
                nc.sync.dma_start(wvc[:], wv_r[:, ec, :])
                wvs.append(wvc)
                xc = persist.tile([128, S], bf16, name=f"x{ec}")
                nc.sync.dma_start(xc[:, 0 : S // 4], xT_r[:, ec, 0 : S // 4])
                xts.append(xc)
            for q in range(1, 4):
                for ec in range(8):
                    nc.sync.dma_start(
                        xts[ec][:, q * S // 4 : (q + 1) * S // 4],
                        xT_r[:, ec, q * S // 4 : (q + 1) * S // 4],
                    )
            wq_sb = persist.tile([128, 8, HK], bf16)
            nc.sync.dma_start(wq_sb[:], wq.ap().rearrange("(ec p) f -> p ec f", p=128))
            wk_sb = persist.tile([128, 8, HK], bf16)
            nc.sync.dma_start(wk_sb[:], wk.ap().rearrange("(ec p) f -> p ec f", p=128))
            mask_sb = persist.tile([128, 2, QC], bf16)
            nc.sync.dma_start(mask_sb[:], masks.ap())
            wo_sb = persist.tile([128, 4, E], bf16)
            nc.sync.dma_start(wo_sb[:], wo.ap().rearrange("(hc p) e -> p hc e", p=128))

            qt_sb = persist.tile([128, 4, S], bf16)
            kt_sb = persist.tile([128, 4, S], bf16)
            ot_sb = persist.tile([128, 4, S], bf16)

            def quarters():
                t1 = ps_shared.tile([128, 4, QC], f32, tag="ps")
                t2 = ps_shared.tile([128, 4, QC], f32, tag="ps")
                return [
                    t1[:, 0:2, :].rearrange("p a b -> p (a b)"),
                    t1[:, 2:4, :].rearrange("p a b -> p (a b)"),
                    t2[:, 0:2, :].rearrange("p a b -> p (a b)"),
                    t2[:, 2:4, :].rearrange("p a b -> p (a b)"),
                ]

            # ---- V projection: V_sb[tok_part, tok_block, head, 64:128]; cols
            # 0:64 are ones so the PV psum puts the prob sums at partitions
            # 0:64 (the custom-DVE reciprocal requires base partition 0)
            v_sb = persist.tile([128, 16, HG, 128], bf16)
            nc.gpsimd.memset(v_sb[:, :, :, 0:64], 1.0)
            for tp in range(8):  # pairs of tok blocks -> two psum halves
                t1 = ps_shared.tile([128, 4, QC], f32, tag="ps")
                halves = [
                    t1[:, 0:2, :].rearrange("p a b -> p (a b)"),
                    t1[:, 2:4, :].rearrange("p a b -> p (a b)"),
                ]
                for ec in range(8):
                    for j in range(2):
                        tb = 2 * tp + j
                        nc.tensor.matmul(
                            halves[j],
                            xts[ec][:, tb * 128 : (tb + 1) * 128],
                            wvs[ec][:],
                            start=(ec == 0),
                            stop=(ec == 7),
                        )
                nc.vector.tensor_copy(
                    v_sb[:, 2 * tp : 2 * tp + 2, :, 64:128],
                    t1[:].rearrange("p a b -> p (a b)").rearrange("p (j h k) -> p j h k", j=2, h=HG),
                )

            def qkt_group(hp, dst, w_sb, tp2, split_copy=False):
                # one projection psum-group: dst[:, hp, tp2-half of tokens]
                t1 = ps_shared.tile([128, 4, QC], f32, tag="ps", name="pj")
                halves = [
                    t1[:, 0:2, :].rearrange("p a b -> p (a b)"),
                    t1[:, 2:4, :].rearrange("p a b -> p (a b)"),
                ]
                for ec in range(8):
                    for j in range(2):
                        tc_ = 2 * tp2 + j
                        nc.tensor.matmul(
                            halves[j],
                            w_sb[:, ec, hp * 128 : (hp + 1) * 128],
                            xts[ec][:, tc_ * 512 : (tc_ + 1) * 512],
                            start=(ec == 0),
                            stop=(ec == 7),
                        )
                if split_copy:
                    # before the first attn the ACT engine is idle:
                    # split the copy across both engines
                    nc.scalar.copy(
                        dst[:, hp, tp2 * 1024 : tp2 * 1024 + 512],
                        t1[:, 0:2, :].rearrange("p a b -> p (a b)"),
                    )
                    nc.vector.tensor_copy(
                        dst[:, hp, tp2 * 1024 + 512 : (tp2 + 1) * 1024],
                        t1[:, 2:4, :].rearrange("p a b -> p (a b)"),
                    )
                else:
                    nc.vector.tensor_copy(
                        dst[:, hp, tp2 * 1024 : (tp2 + 1) * 1024],
                        t1[:].rearrange("p a b -> p (a b)"),
                    )

            # group order: K first (attention needs all of K immediately),
            # then Q with the second token half first (t=3 consumed first)
            qkt_groups = (
                (kt_sb, wk_sb, 0),
                (kt_sb, wk_sb, 1),
                (qt_sb, wq_sb, 1),
                (qt_sb, wq_sb, 0),
            )

            def attn(h, t):
                hp, hh = divmod(h, 2)
                pb = 64 * hh
                qc0, qc1 = 2 * t, 2 * t + 1
                mb0, mb1 = 4 * t + 2, 4 * t + 4
                # ptab[:, mb, 0, :] = probs for qc0, [:, mb, 1, :] = qc1
                ptab = pt_pool.tile([128, 16, 2, QC], bf16, tag="pt")
                for g0 in range(0, mb1, 2):
                    pl = ps_shared.tile([128, 4, QC], f32, tag="ps", name="pl")
                    for i in (0, 1):
                        mb = g0 + i
                        if mb < mb0:
                            # both q-chunks in one 512-wide matmul
                            nc.tensor.matmul(
                                pl[:, 2 * i : 2 * i + 2, :].rearrange(
                                    "p a b -> p (a b)"
                                ),
                                kt_sb[pb : pb + 64, hp, mb * 128 : (mb + 1) * 128],
                                qt_sb[pb : pb + 64, hp, qc0 * QC : (qc0 + 2) * QC],
                                start=True,
                                stop=True,
                            )
                        elif mb < mb1 - 1:
                            nc.tensor.matmul(
                                pl[:, 2 * i + 1, :],
                                kt_sb[pb : pb + 64, hp, mb * 128 : (mb + 1) * 128],
                                qt_sb[pb : pb + 64, hp, qc1 * QC : (qc1 + 1) * QC],
                                start=True,
                                stop=True,
                            )
                        else:
                            # last diagonal block: q-block qc1-lo is entirely
                            # above the diagonal, so only compute the hi half
                            # (the mask TT zeroes the stale lo half of ptab)
                            nc.tensor.matmul(
                                pl[:, 2 * i + 1, 128:QC],
                                kt_sb[pb : pb + 64, hp, mb * 128 : (mb + 1) * 128],
                                qt_sb[
                                    pb : pb + 64,
                                    hp,
                                    qc1 * QC + 128 : (qc1 + 1) * QC,
                                ],
                                start=True,
                                stop=True,
                            )
                    if g0 < mb0:
                        nc.scalar.activation(
                            ptab[:, g0 : g0 + 2, :, :],
                            pl[:].rearrange("p a b -> p (a b)").rearrange(
                                "p (i j b) -> p i j b", i=2, j=2
                            ),
                            Exp,
                            scale=0.125,
                        )
                    else:
                        nc.scalar.activation(
                            ptab[:, g0 : g0 + 2, 1, :],
                            pl[:, 1::2, :],
                            Exp,
                            scale=0.125,
                        )
                # causal 0/1 masks on the diagonal-band blocks
                nc.vector.tensor_tensor(
                    ptab[:, mb0 - 2 : mb0, 0, :],
                    ptab[:, mb0 - 2 : mb0, 0, :],
                    mask_sb[:],
                    mult,
                )
                nc.vector.tensor_tensor(
                    ptab[:, mb1 - 2 : mb1, 1, :],
                    ptab[:, mb1 - 2 : mb1, 1, :],
                    mask_sb[:],
                    mult,
                )
                # PV: psum rows 0:64 = prob sums, rows 64:128 = O^T.
                # po is one 2KB psum zero-region: start once, stop at the end;
                # sub-diagonal blocks do both q-chunks in one 512-wide matmul.
                po = ps_pv.tile([128, 2, QC], f32, tag="pv")
                for mb in range(mb1):
                    if mb < mb0:
                        nc.tensor.matmul(
                            po[:].rearrange("p a b -> p (a b)"),
                            v_sb[:, mb, h, :],
                            ptab[:, mb, :, :].rearrange("p a b -> p (a b)"),
                            start=(mb == 0),
                            stop=False,
                            skip_group_check=True,
                        )
                    elif mb < mb1 - 1:
                        nc.tensor.matmul(
                            po[:, 1, :],
                            v_sb[:, mb, h, :],
                            ptab[:, mb, 1, :],
                            start=False,
                            stop=False,
                            skip_group_check=True,
                        )
                    else:
                        nc.tensor.matmul(
                            po[:, 1, 128:QC],
                            v_sb[:, mb, h, :],
                            ptab[:, mb, 1, 128:QC],
                            start=False,
                            stop=True,
                            skip_group_check=True,
                        )
                for j, qc in ((0, qc0), (1, qc1)):
                    # prob sums are always normal positive fp32, so the fast
                    # approximate reciprocal (~18 bits, single DVE op) is safe
                    bc = small.tile([64, QC], f32, tag="bc")
                    nc.vector.reciprocal_approx_fast(bc[:], po[0:64, j, :])
                    nc.vector.tensor_tensor(
                        ot_sb[pb : pb + 64, hp, qc * QC : (qc + 1) * QC],
                        po[64:128, j, :],
                        bc[:],
                        mult,
                    )

            def outproj(tc_, tail=False):
                # y^T[eb-pair, tok chunk tc_] for all eb; copies alternate
                # between scalar and vector engines
                for ep in range(4):
                    t1 = ps_shared.tile([128, 4, QC], f32, tag="ps", name="po2")
                    halves = [
                        t1[:, 0:2, :].rearrange("p a b -> p (a b)"),
                        t1[:, 2:4, :].rearrange("p a b -> p (a b)"),
                    ]
                    for hc in range(4):
                        for j in range(2):
                            eb = 2 * ep + j
                            nc.tensor.matmul(
                                halves[j],
                                wo_sb[:, hc, eb * 128 : (eb + 1) * 128],
                                ot_sb[:, hc, tc_ * 512 : (tc_ + 1) * 512],
                                start=(hc == 0),
                                stop=(hc == 3),
                            )
                    ysb = ypool.tile([128, 2, 512], bf16, tag="ysb")
                    t1v = t1[:].rearrange("p a b -> p (a b)").rearrange(
                        "p (j b) -> p j b", j=2
                    )
                    if tail and ep == 3:
                        # final block: split across both copy engines so the
                        # post-PE critical path is one half-copy + DMA
                        nc.scalar.copy(ysb[:, 0, :], t1v[:, 0, :])
                        nc.vector.tensor_copy(ysb[:, 1, :], t1v[:, 1, :])
                    else:
                        # keep copies off the ACT engine: during the hp=3
                        # stretch the exp chain is the local bottleneck
                        nc.vector.tensor_copy(ysb[:], t1v[:])
                    yv = yT.ap().rearrange("(eo p) t -> p eo t", p=128)
                    nc.sync.dma_start(
                        yv[:, 2 * ep : 2 * ep + 2, tc_ * 512 : (tc_ + 1) * 512], ysb[:]
                    )

            for dst, w_sb, tp2 in qkt_groups:
                qkt_group(0, dst, w_sb, tp2, split_copy=True)
            for hp in range(4):
                for t in range(NP - 1, -1, -1):
                    attn(2 * hp, t)
                    attn(2 * hp + 1, t)
                    if hp == 3 and t < NP - 1:
                        # chunk t+1 completed a whole attn call ago, so its
                        # output projection starts without waiting on the
                        # just-issued normalize TTs
                        outproj(t + 1)
                if hp < 3:
                    for dst, w_sb, tp2 in qkt_groups:
                        qkt_group(hp + 1, dst, w_sb, tp2)
            outproj(0, tail=True)

    from concourse.library_overlay import lower_extended_insts

    lower_extended_insts(nc)  # populate .instr for the custom-DVE reciprocal
    _dedup_ldweights(nc)
    if legalize:
        _legalize_sync_waits(nc)
    return nc


def _make_masks():
    q = np.arange(QC)[None, :]
    m = np.arange(128)[:, None]
    mask_a = (q >= m).astype(BF16)
    mask_b = (q >= m + 128).astype(BF16)
    return np.stack([mask_a, mask_b], axis=1)  # [128, 2, QC]


def kernel(x_BSE, w_q, w_k, w_v, w_o):
    global _compiled
    from concourse.bass_utils import run_bass_kernel_spmd

    x_BSE = np.asarray(x_BSE)
    w_q, w_k = np.asarray(w_q), np.asarray(w_k)
    w_v, w_o = np.asarray(w_v), np.asarray(w_o)
    B = x_BSE.shape[0]
    if _compiled is None:
        _compiled = _build_nc()
    nc = _compiled

    masks = _make_masks()
    in_maps = []
    for b in range(B):
        xTb = np.ascontiguousarray(x_BSE[b].astype(BF16).T)  # [E, S]
        for g in range(2):
            hsl = slice(g * HG, (g + 1) * HG)
            in_maps.append(
                {
                    "xT": xTb,
                    "wq": np.ascontiguousarray(
                        w_q[:, hsl, :].reshape(E, HK).astype(BF16)
                    ),
                    "wk": np.ascontiguousarray(
                        w_k[:, hsl, :].reshape(E, HK).astype(BF16)
                    ),
                    "wv": np.ascontiguousarray(
                        w_v[:, hsl, :].reshape(E, HK).astype(BF16)
                    ),
                    "wo": np.ascontiguousarray(
                        w_o[:, hsl, :].reshape(E, HK).astype(BF16).T
                    ),
                    "masks": masks,
                }
            )

    res = run_bass_kernel_spmd(nc, in_maps, core_ids=list(range(2 * B)))

    out = np.empty((B, S, E), dtype=np.float32)
    for b in range(B):
        acc = res.results[2 * b]["yT"].astype(np.float32) + res.results[2 * b + 1][
            "yT"
        ].astype(np.float32)
        out[b] = acc.T
    return out



# revision 38
# speedup vs baseline: 1.0003x; 1.0003x over previous
"""Causal multi-head attention (B=4, S=2048, E=1024, H=16, K=64, fp32) on 8 TRN2 cores.

Sharding: core c = 2*b + g handles batch b and head-group g (8 of 16 heads).
Each core computes QKV projections for its heads, causal flash-style
attention, and a partial output projection; the host sums the two
head-group partials per batch.

Layout choices (all on-chip, per core):
  - Projections produce Q^T, K^T directly ([hk, tok], hk on partitions) so
    QK^T needs no transposes; logits are computed transposed ([m, q], keys on
    partitions), which also lets PV consume the probs tile directly as the
    matmul moving side with no transpose anywhere.
  - Softmax skips max-subtraction (logits ~ N(0,1); exp is safe in fp32) and
    gets the normalizer for free from 64 ones-columns appended to V: the PV
    matmul writes O^T to psum rows 0:64 and replicates the per-q prob-sums
    across rows 64:128, so normalization is a plain reciprocal+multiply with
    no cross-partition broadcast.
  - Causal masking multiplies the two diagonal-band blocks of each q-chunk by
    a 0/1 mask after exp.
  - All matmuls run in bf16 (fp32 PSUM accumulate).
  - q-chunks are processed in pairs and projections are loop-ordered so that
    consecutive PE matmuls share their stationary operand; a post-pass
    removes the duplicate LDWEIGHTS.  Projections for head-pair hp+1 are
    emitted between attention chunks of hp so the (ACT-bound) softmax
    stretches keep the PE busy and the HAM clock-gate stays warm.
"""

import numpy as np
import ml_dtypes

S = 2048          # sequence length
E = 1024          # model dim
HG = 8            # heads per core
HK = HG * 64      # packed head*K = 512
QC = 256          # q-chunk
NP = S // (2 * QC)  # 4 q-chunk pairs
BF16 = ml_dtypes.bfloat16

_compiled = None


def _legalize_sync_waits(nc, max_other=1, max_evsem=2):
    """Walrus in this toolchain caps sync-wait commands per instruction; split
    excess waits onto standalone InstNoOps on the same engine."""
    import concourse.mybir as mybir

    for fn in nc.m.functions:
        for bb in fn.blocks:
            new = []
            changed = False
            for inst in bb.instructions:
                si = inst.sync_info
                ow = list(si.on_wait) if (si is not None and si.on_wait) else []
                cap = (
                    max_evsem
                    if type(inst).__name__ == "InstEventSemaphore"
                    else max_other
                )
                if len(ow) > cap:
                    extras, kept = ow[:-cap], ow[-cap:]
                    for i in range(0, len(extras), max_other):
                        new.append(
                            mybir.InstNoOp(
                                name=f"{inst.name}-waitsplit-{i}",
                                sync_info=mybir.SyncInfo(
                                    on_wait=extras[i : i + max_other], on_update=[]
                                ),
                                engine=inst.engine,
                                bass_nofuse=True,
                            )
                        )
                    si.on_wait = kept
                    changed = True
                new.append(inst)
            if changed:
                bb.instructions = new


def _dedup_ldweights(nc):
    """Remove an InstLdweights whose weight AP is identical to the immediately
    preceding PE weight load (pattern LDW A, MM, LDW A, MM ...).  All dedup
    targets here load from persistent SBUF tensors, so there is no WAR hazard;
    any waits/updates on the removed load migrate to the next PE
    instruction."""
    import concourse.mybir as mybir
    import re as _re

    n = 0
    for fn in nc.m.functions:
        for bb in fn.blocks:
            insts = bb.instructions
            last_key = None
            drop = set()
            pend_w, pend_u = [], []
            for idx, inst in enumerate(insts):
                eng = getattr(inst, "engine", None)
                if eng != mybir.EngineType.PE:
                    continue
                nm = type(inst).__name__
                if pend_w or pend_u:
                    si = inst.sync_info
                    w = (list(si.on_wait or []) if si else []) + pend_w
                    u = (list(si.on_update or []) if si else []) + pend_u
                    inst.sync_info = mybir.SyncInfo(on_wait=w, on_update=u)
                    pend_w, pend_u = [], []
                if nm == "InstLdweights":
                    js = mybir.instruction_to_pretty_json_string(inst)
                    key = _re.sub(r'"name":\s*"[^"]*",?', "", js)
                    key = _re.sub(r'"sync_info":\s*\{.*?\n  \}', "", key, flags=_re.S)
                    key = _re.sub(r'"debug[^"]*":\s*(\{.*?\n  \}|"[^"]*"|null),?', "", key, flags=_re.S)
                    if key == last_key:
                        si = inst.sync_info
                        if si is not None:
                            pend_w += list(si.on_wait or [])
                            pend_u += list(si.on_update or [])
                        drop.add(idx)
                        n += 1
                    else:
                        last_key = key
                elif nm != "InstMatmult":
                    last_key = None
            if drop:
                bb.instructions = [i for j, i in enumerate(insts) if j not in drop]
    return n


def _build_nc(legalize=True):
    import concourse.bass as bass
    import concourse.mybir as mybir
    import concourse.tile as tile

    bf16 = mybir.dt.bfloat16
    f32 = mybir.dt.float32
    Exp = mybir.ActivationFunctionType.Exp
    mult = mybir.AluOpType.mult

    nc = bass.Bass("TRN2", target_bir_lowering=False, debug=False)

    xT = nc.dram_tensor("xT", [E, S], bf16, kind="ExternalInput")
    wq = nc.dram_tensor("wq", [E, HK], bf16, kind="ExternalInput")
    wk = nc.dram_tensor("wk", [E, HK], bf16, kind="ExternalInput")
    wv = nc.dram_tensor("wv", [E, HK], bf16, kind="ExternalInput")
    wo = nc.dram_tensor("wo", [HK, E], bf16, kind="ExternalInput")
    masks = nc.dram_tensor("masks", [128, 2, QC], bf16, kind="ExternalInput")
    yT = nc.dram_tensor("yT", [E, S], bf16, kind="ExternalOutput")

    with tile.TileContext(nc) as tc:
        with (
            tc.tile_pool(name="persist", bufs=1) as persist,
            tc.tile_pool(name="pt", bufs=3) as pt_pool,
            tc.tile_pool(name="small", bufs=6) as small,
            tc.tile_pool(name="ysb", bufs=2) as ypool,
            tc.tile_pool(name="ps", bufs=3, space="PSUM") as ps_shared,
            tc.tile_pool(name="ps_pv", bufs=2, space="PSUM") as ps_pv,
        ):
            # ---- load inputs ----
            # All input DMAs share the sync-engine HW queue, so they transfer
            # in issue order: interleave the wv/x chunks the V projection
            # consumes first, then wq/wk (needed ~25us in), masks, and wo
            # (needed only ~170us in).  This lets the PE start ~2us after
            # launch instead of waiting for the whole 8MB input load.
            xT_r = xT.ap().rearrange("(ec p) t -> p ec t", p=128)
            wv_r = wv.ap().rearrange("(ec p) f -> p ec f", p=128)
            xts = []
            wvs = []
            for ec in range(8):
                wvc = persist.tile([128, HK], bf16, name=f"wv{ec}")
                nc.sync.dma_start(wvc[:], wv_r[:, ec, :])
                wvs.append(wvc)
                xc = persist.tile([128, S], bf16, name=f"x{ec}")
                nc.sync.dma_start(xc[:, 0 : S // 4], xT_r[:, ec, 0 : S // 4])
                xts.append(xc)
            for q in range(1, 4):
                for ec in range(8):
                    nc.sync.dma_start(
                        xts[ec][:, q * S // 4 : (q + 1) * S // 4],
                        xT_r[:, ec, q * S // 4 : (q + 1) * S // 4],
                    )
            wq_sb = persist.tile([128, 8, HK], bf16)
            nc.sync.dma_start(wq_sb[:], wq.ap().rearrange("(ec p) f -> p ec f", p=128))
            wk_sb = persist.tile([128, 8, HK], bf16)
            nc.sync.dma_start(wk_sb[:], wk.ap().rearrange("(ec p) f -> p ec f", p=128))
            mask_sb = persist.tile([128, 2, QC], bf16)
            nc.sync.dma_start(mask_sb[:], masks.ap())
            wo_sb = persist.tile([128, 4, E], bf16)
            nc.sync.dma_start(wo_sb[:], wo.ap().rearrange("(hc p) e -> p hc e", p=128))

            qt_sb = persist.tile([128, 4, S], bf16)
            kt_sb = persist.tile([128, 4, S], bf16)
            ot_sb = persist.tile([128, 4, S], bf16)

            def quarters():
                t1 = ps_shared.tile([128, 4, QC], f32, tag="ps")
                t2 = ps_shared.tile([128, 4, QC], f32, tag="ps")
                return [
                    t1[:, 0:2, :].rearrange("p a b -> p (a b)"),
                    t1[:, 2:4, :].rearrange("p a b -> p (a b)"),
                    t2[:, 0:2, :].rearrange("p a b -> p (a b)"),
                    t2[:, 2:4, :].rearrange("p a b -> p (a b)"),
                ]

            # ---- V projection: V_sb[tok_part, tok_block, head, 64:128]; cols
            # 0:64 are ones so the PV psum puts the prob sums at partitions
            # 0:64 (the custom-DVE reciprocal requires base partition 0)
            v_sb = persist.tile([128, 16, HG, 128], bf16)
            nc.gpsimd.memset(v_sb[:, :, :, 0:64], 1.0)
            for tp in range(8):  # pairs of tok blocks -> two psum halves
                t1 = ps_shared.tile([128, 4, QC], f32, tag="ps")
                halves = [
                    t1[:, 0:2, :].rearrange("p a b -> p (a b)"),
                    t1[:, 2:4, :].rearrange("p a b -> p (a b)"),
                ]
                for ec in range(8):
                    for j in range(2):
                        tb = 2 * tp + j
                        nc.tensor.matmul(
                            halves[j],
                            xts[ec][:, tb * 128 : (tb + 1) * 128],
                            wvs[ec][:],
                            start=(ec == 0),
                            stop=(ec == 7),
                        )
                nc.vector.tensor_copy(
                    v_sb[:, 2 * tp : 2 * tp + 2, :, 64:128],
                    t1[:].rearrange("p a b -> p (a b)").rearrange("p (j h k) -> p j h k", j=2, h=HG),
                )

            def qkt_group(hp, dst, w_sb, tp2, split_copy=False):
                # one projection psum-group: dst[:, hp, tp2-half of tokens]
                t1 = ps_shared.tile([128, 4, QC], f32, tag="ps", name="pj")
                halves = [
                    t1[:, 0:2, :].rearrange("p a b -> p (a b)"),
                    t1[:, 2:4, :].rearrange("p a b -> p (a b)"),
                ]
                for ec in range(8):
                    for j in range(2):
                        tc_ = 2 * tp2 + j
                        nc.tensor.matmul(
                            halves[j],
                            w_sb[:, ec, hp * 128 : (hp + 1) * 128],
                            xts[ec][:, tc_ * 512 : (tc_ + 1) * 512],
                            start=(ec == 0),
                            stop=(ec == 7),
                        )
                if split_copy:
                    # before the first attn the ACT engine is idle: split the
                    # copy across both engines, with the LATER tokens (what
                    # attn consumes first, t descending) on the faster-to-
                    # drain scalar engine
                    nc.scalar.copy(
                        dst[:, hp, tp2 * 1024 + 512 : (tp2 + 1) * 1024],
                        t1[:, 2:4, :].rearrange("p a b -> p (a b)"),
                    )
                    nc.vector.tensor_copy(
                        dst[:, hp, tp2 * 1024 : tp2 * 1024 + 512],
                        t1[:, 0:2, :].rearrange("p a b -> p (a b)"),
                    )
                else:
                    nc.vector.tensor_copy(
                        dst[:, hp, tp2 * 1024 : (tp2 + 1) * 1024],
                        t1[:].rearrange("p a b -> p (a b)"),
                    )

            # group order: K first (attention needs all of K immediately),
            # then Q with the second token half first (t=3 consumed first)
            qkt_groups = (
                (kt_sb, wk_sb, 0),
                (kt_sb, wk_sb, 1),
                (qt_sb, wq_sb, 1),
                (qt_sb, wq_sb, 0),
            )

            def attn(h, t):
                hp, hh = divmod(h, 2)
                pb = 64 * hh
                qc0, qc1 = 2 * t, 2 * t + 1
                mb0, mb1 = 4 * t + 2, 4 * t + 4
                # ptab[:, mb, 0, :] = probs for qc0, [:, mb, 1, :] = qc1
                ptab = pt_pool.tile([128, 16, 2, QC], bf16, tag="pt")
                for g0 in range(0, mb1, 2):
                    pl = ps_shared.tile([128, 4, QC], f32, tag="ps", name="pl")
                    for i in (0, 1):
                        mb = g0 + i
                        if mb < mb0:
                            # both q-chunks in one 512-wide matmul
                            nc.tensor.matmul(
                                pl[:, 2 * i : 2 * i + 2, :].rearrange(
                                    "p a b -> p (a b)"
                                ),
                                kt_sb[pb : pb + 64, hp, mb * 128 : (mb + 1) * 128],
                                qt_sb[pb : pb + 64, hp, qc0 * QC : (qc0 + 2) * QC],
                                start=True,
                                stop=True,
                            )
                        elif mb < mb1 - 1:
                            nc.tensor.matmul(
                                pl[:, 2 * i + 1, :],
                                kt_sb[pb : pb + 64, hp, mb * 128 : (mb + 1) * 128],
                                qt_sb[pb : pb + 64, hp, qc1 * QC : (qc1 + 1) * QC],
                                start=True,
                                stop=True,
                            )
                        else:
                            # last diagonal block: q-block qc1-lo is entirely
                            # above the diagonal, so only compute the hi half
                            # (the mask TT zeroes the stale lo half of ptab)
                            nc.tensor.matmul(
                                pl[:, 2 * i + 1, 128:QC],
                                kt_sb[pb : pb + 64, hp, mb * 128 : (mb + 1) * 128],
                                qt_sb[
                                    pb : pb + 64,
                                    hp,
                                    qc1 * QC + 128 : (qc1 + 1) * QC,
                                ],
                                start=True,
                                stop=True,
                            )
                    if g0 < mb0:
                        nc.scalar.activation(
                            ptab[:, g0 : g0 + 2, :, :],
                            pl[:].rearrange("p a b -> p (a b)").rearrange(
                                "p (i j b) -> p i j b", i=2, j=2
                            ),
                            Exp,
                            scale=0.125,
                        )
                    else:
                        nc.scalar.activation(
                            ptab[:, g0 : g0 + 2, 1, :],
                            pl[:, 1::2, :],
                            Exp,
                            scale=0.125,
                        )
                # causal 0/1 masks on the diagonal-band blocks
                nc.vector.tensor_tensor(
                    ptab[:, mb0 - 2 : mb0, 0, :],
                    ptab[:, mb0 - 2 : mb0, 0, :],
                    mask_sb[:],
                    mult,
                )
                nc.vector.tensor_tensor(
                    ptab[:, mb1 - 2 : mb1, 1, :],
                    ptab[:, mb1 - 2 : mb1, 1, :],
                    mask_sb[:],
                    mult,
                )
                # PV: psum rows 0:64 = prob sums, rows 64:128 = O^T.
                # po is one 2KB psum zero-region: start once, stop at the end;
                # sub-diagonal blocks do both q-chunks in one 512-wide matmul.
                po = ps_pv.tile([128, 2, QC], f32, tag="pv")
                for mb in range(mb1):
                    if mb < mb0:
                        nc.tensor.matmul(
                            po[:].rearrange("p a b -> p (a b)"),
                            v_sb[:, mb, h, :],
                            ptab[:, mb, :, :].rearrange("p a b -> p (a b)"),
                            start=(mb == 0),
                            stop=False,
                            skip_group_check=True,
                        )
                    elif mb < mb1 - 1:
                        nc.tensor.matmul(
                            po[:, 1, :],
                            v_sb[:, mb, h, :],
                            ptab[:, mb, 1, :],
                            start=False,
                            stop=False,
                            skip_group_check=True,
                        )
                    else:
                        nc.tensor.matmul(
                            po[:, 1, 128:QC],
                            v_sb[:, mb, h, :],
                            ptab[:, mb, 1, 128:QC],
                            start=False,
                            stop=True,
                            skip_group_check=True,
                        )
                for j, qc in ((0, qc0), (1, qc1)):
                    # prob sums are always normal positive fp32, so the fast
                    # approximate reciprocal (~18 bits, single DVE op) is safe
                    bc = small.tile([64, QC], f32, tag="bc")
                    nc.vector.reciprocal_approx_fast(bc[:], po[0:64, j, :])
                    nc.vector.tensor_tensor(
                        ot_sb[pb : pb + 64, hp, qc * QC : (qc + 1) * QC],
                        po[64:128, j, :],
                        bc[:],
                        mult,
                    )

            def outproj(tc_, tail=False):
                # y^T[eb-pair, tok chunk tc_] for all eb; copies alternate
                # between scalar and vector engines
                for ep in range(4):
                    t1 = ps_shared.tile([128, 4, QC], f32, tag="ps", name="po2")
                    halves = [
                        t1[:, 0:2, :].rearrange("p a b -> p (a b)"),
                        t1[:, 2:4, :].rearrange("p a b -> p (a b)"),
                    ]
                    for hc in range(4):
                        for j in range(2):
                            eb = 2 * ep + j
                            nc.tensor.matmul(
                                halves[j],
                                wo_sb[:, hc, eb * 128 : (eb + 1) * 128],
                                ot_sb[:, hc, tc_ * 512 : (tc_ + 1) * 512],
                                start=(hc == 0),
                                stop=(hc == 3),
                            )
                    ysb = ypool.tile([128, 2, 512], bf16, tag="ysb")
                    t1v = t1[:].rearrange("p a b -> p (a b)").rearrange(
                        "p (j b) -> p j b", j=2
                    )
                    if tail and ep == 3:
                        # final block: split across both copy engines so the
                        # post-PE critical path is one half-copy + DMA
                        nc.scalar.copy(ysb[:, 0, :], t1v[:, 0, :])
                        nc.vector.tensor_copy(ysb[:, 1, :], t1v[:, 1, :])
                    else:
                        # keep copies off the ACT engine: during the hp=3
                        # stretch the exp chain is the local bottleneck
                        nc.vector.tensor_copy(ysb[:], t1v[:])
                    yv = yT.ap().rearrange("(eo p) t -> p eo t", p=128)
                    nc.sync.dma_start(
                        yv[:, 2 * ep : 2 * ep + 2, tc_ * 512 : (tc_ + 1) * 512], ysb[:]
                    )

            for dst, w_sb, tp2 in qkt_groups:
                qkt_group(0, dst, w_sb, tp2, split_copy=True)
            for hp in range(4):
                for t in range(NP - 1, -1, -1):
                    attn(2 * hp, t)
                    attn(2 * hp + 1, t)
                    if hp == 3 and t < NP - 1:
                        # chunk t+1 completed a whole attn call ago, so its
                        # output projection starts without waiting on the
                        # just-issued normalize TTs
                        outproj(t + 1)
                if hp < 3:
                    for dst, w_sb, tp2 in qkt_groups:
                        qkt_group(hp + 1, dst, w_sb, tp2)
            outproj(0, tail=True)

    from concourse.library_overlay import lower_extended_insts

    lower_extended_insts(nc)  # populate .instr for the custom-DVE reciprocal
    _dedup_ldweights(nc)
    if legalize:
        _legalize_sync_waits(nc)
    return nc


def _make_masks():
    q = np.arange(QC)[None, :]
    m = np.arange(128)[:, None]
    mask_a = (q >= m).astype(BF16)
    mask_b = (q >= m + 128).astype(BF16)
    return np.stack([mask_a, mask_b], axis=1)  # [128, 2, QC]


def kernel(x_BSE, w_q, w_k, w_v, w_o):
    global _compiled
    from concourse.bass_utils import run_bass_kernel_spmd

    x_BSE = np.asarray(x_BSE)
    w_q, w_k = np.asarray(w_q), np.asarray(w_k)
    w_v, w_o = np.asarray(w_v), np.asarray(w_o)
    B = x_BSE.shape[0]
    if _compiled is None:
        _compiled = _build_nc()
    nc = _compiled

    masks = _make_masks()
    in_maps = []
    for b in range(B):
        xTb = np.ascontiguousarray(x_BSE[b].astype(BF16).T)  # [E, S]
        for g in range(2):
            hsl = slice(g * HG, (g + 1) * HG)
            in_maps.append(
                {
                    "xT": xTb,
                    "wq": np.ascontiguousarray(
                        w_q[:, hsl, :].reshape(E, HK).astype(BF16)
                    ),
                    "wk": np.ascontiguousarray(
                        w_k[:, hsl, :].reshape(E, HK).astype(BF16)
                    ),
                    "wv": np.ascontiguousarray(
                        w_v[:, hsl, :].reshape(E, HK).astype(BF16)
                    ),
                    "wo": np.ascontiguousarray(
                        w_o[:, hsl, :].reshape(E, HK).astype(BF16).T
                    ),
                    "masks": masks,
                }
            )

    res = run_bass_kernel_spmd(nc, in_maps, core_ids=list(range(2 * B)))

    out = np.empty((B, S, E), dtype=np.float32)
    for b in range(B):
        acc = res.results[2 * b]["yT"].astype(np.float32) + res.results[2 * b + 1][
            "yT"
        ].astype(np.float32)
        out[b] = acc.T
    return out

